# revision 17
# baseline (speedup 1.0000x reference)
"""Self-contained Trainium2 kernel: block-circulant FFT linear layer.

out = ifft(fft(x_blocks) * fft(W)).real summed over input blocks, computed as
three PE matmul stages (real-FFT basis, per-frequency block matmul, inverse
real-FFT).  v3 design:

- fwd stage in fp32r (full PE rate at moving free dim 512); spectral data is
  cast to bf16 at PSUM evacuation; mid + inv stages run bf16 x bf16
  (1 cycle/row at any free dim -> small 128-batch windows pipeline finely)
- the two inter-stage shuffles are DVE 32x32 StreamTransposes on *u32-packed
  bf16 pairs* (two batch rows per element), halving DVE transpose work
- one shared 4-bank PSUM ring for all three stages -> few, large (2048-elem)
  evacuation copies, split between ACT and DVE
- flat per-partition-contiguous DMA layouts (128 descriptors per transfer);
  inputs on the sync HWDGE ring, outputs + consts on the gpsimd SWDGE ring.

SPMD over 8 NeuronCores (batch-sharded), 512 batch rows per core.

kernel(x, W): x [4096, 4096] f32, W [64, 64, 64] f32 -> [4096, 4096] f32.
"""
import numpy as np
import ml_dtypes
import concourse.bass as bass
import concourse.bacc as bacc
import concourse.mybir as mybir
import concourse.tile as tile
from concourse.bass_utils import run_bass_kernel_spmd

N_CORES = 8
B, IN, OUT, BS = 4096, 4096, 4096, 64
BC = B // N_CORES            # 512 batch rows per core
NK = 32                      # bin tiles (tile 0 carries bins 0 and 32)
NA = 32                      # jpair / ipair tiles
F32 = mybir.dt.float32
F32R = mybir.dt.float32r
BF16 = mybir.dt.bfloat16
F16 = mybir.dt.float16

# tunables
BW = 64                      # batch window
NH = BC // BW                # windows per core
W2 = BW // 2                 # u32-packed pair count per window
IN_SPLIT = 1                 # input DMA chunks per window
WARMUP = 12                  # PE warmup matmuls before first window
# evac engine per psum tile: 'v' = DVE, 's' = ACT
EV_FWD = ("s", "s", "s", "v")
EV_MID = ("s", "s", "v", "s")
EV_INV = ("v", "s", "v", "s")


# ---------------- host-side constant matrices ----------------

def make_fmat():
    t = np.arange(BS)[:, None]
    c = np.arange(BS)[None, :]
    k = np.where(c <= 32, c, c - 32)
    ang = 2 * np.pi * k * t / BS
    F = np.where(c <= 32, np.cos(ang), np.sin(ang))
    bd = np.zeros((128, 128), np.float32)
    bd[:64, :64] = F
    bd[64:, 64:] = F
    return bd.astype(np.float16)


def make_gmat():
    tau = np.arange(BS)[None, :]
    c = np.arange(BS)[:, None]
    k = np.where(c <= 32, c, c - 32)
    ang = 2 * np.pi * k * tau / BS
    base = np.where(c <= 32, np.cos(ang), np.sin(ang))
    scale = np.where((c % 32) == 0, 1.0 / BS, 2.0 / BS)
    G = base * scale
    bd = np.zeros((128, 128), np.float32)
    bd[:64, :64] = G
    bd[64:, 64:] = G
    return bd.astype(np.float16)


def make_wmats(W):
    s = np.arange(BS)
    k = np.arange(33)
    ang = 2 * np.pi * k[:, None] * s[None, :] / BS
    wr = np.einsum("ijs,ks->ijk", W, np.cos(ang))
    wi = np.einsum("ijs,ks->ijk", W, np.sin(ang))
    M = np.zeros((NK, 128, 128), np.float32)
    icol = np.empty(64, np.int64)
    for i in range(64):
        a, par = divmod(i, 2)
        icol[i] = 64 * par + a
    for kk in range(NK):
        if kk == 0:
            W32 = wr[:, :, 32]
        Wr, Wi = wr[:, :, kk], wi[:, :, kk]
        for par_j in range(2):
            jrows = np.arange(32) * 2 + par_j
            rre = 64 * par_j + np.arange(32)
            rim = rre + 32
            for i in range(64):
                cre = icol[i]
                cim = cre + 32
                if kk == 0:
                    M[0, rre, cre] = wr[i, jrows, 0]
                    M[0, rim, cim] = W32[i, jrows]
                else:
                    M[kk, rre, cre] = Wr[i, jrows]
                    M[kk, rim, cre] = -Wi[i, jrows]
                    M[kk, rre, cim] = Wi[i, jrows]
                    M[kk, rim, cim] = Wr[i, jrows]
    # device layout: [row r, tile k, col m]
    return np.ascontiguousarray(M.transpose(1, 0, 2)).astype(np.float16)


def prep_x(x):
    """[B, 4096] -> per-core [NH, IN_SPLIT, 128, 32*BW/IN_SPLIT] flat;
    p = par*64+t, j = 2a+par, w local within window."""
    xr = x.reshape(N_CORES, NH, BW, 32, 2, 64)  # [c, h, w, a, par, t]
    xp = xr.transpose(0, 1, 4, 5, 3, 2)  # [c, h, par, t, a, w]
    xp = xp.reshape(N_CORES, NH, 128, 32, IN_SPLIT, BW // IN_SPLIT)
    xp = xp.transpose(0, 1, 4, 2, 3, 5)  # [c, h, s, p, a, w]
    return np.ascontiguousarray(xp).astype(np.float16).reshape(
        N_CORES, NH, IN_SPLIT, 128, 32 * BW // IN_SPLIT)


def post_y(ys):
    """per-core [NH, 128, W2*32*2] -> [B, 4096]; p = par*64+tau, i = 2a+par,
    batch row = c*BC + h*BW + 2*w2 + wp."""
    y = np.stack(ys).astype(np.float32)  # [c, NH, 128, W2*NA*2]
    y = y.reshape(N_CORES, NH, 2, 64, W2, NA, 2)  # [c,h,par,tau,w2,a,wp]
    y = y.transpose(0, 1, 4, 6, 5, 2, 3)  # [c,h,w2,wp,a,par,tau]
    return np.ascontiguousarray(y).reshape(B, OUT)


# ---------------- device kernel ----------------

def build_nc(reps=1):
    nc = bacc.Bacc("TRN2", target_bir_lowering=False, debug=False,
                   num_devices=N_CORES, dynamic_dma_scratch_size=8192)
    XS = 32 * BW // IN_SPLIT
    x_in = nc.dram_tensor("x", [NH, IN_SPLIT, 128, XS], F16,
                          kind="ExternalInput")
    fmat = nc.dram_tensor("fmat", [128, 128], F16, kind="ExternalInput")
    gmat = nc.dram_tensor("gmat", [128, 128], F16, kind="ExternalInput")
    wmat = nc.dram_tensor("wmat", [128, NK * 128], F16, kind="ExternalInput")
    y_out = nc.dram_tensor("y", [NH, 128, W2 * NA * 2], F16,
                           kind="ExternalOutput")

    def ev(engine):
        return nc.vector.tensor_copy if engine == "v" else nc.scalar.copy

    with tile.TileContext(nc) as tc:
        with (
            tc.tile_pool(name="consts", bufs=1) as cpool,
            tc.tile_pool(name="px", bufs=4) as px,    # x in chunks
            tc.tile_pool(name="ps_", bufs=2) as ps_,  # s_sb (packed bf16)
            tc.tile_pool(name="ps2", bufs=2) as ps2_,
            tc.tile_pool(name="po", bufs=2) as po_,
            tc.tile_pool(name="pv", bufs=2) as pv_,
            tc.tile_pool(name="py", bufs=3) as py_,   # y windows
            tc.tile_pool(name="fps", bufs=2, space="PSUM") as fps,
            tc.tile_pool(name="mps", bufs=2, space="PSUM") as mps,
            tc.tile_pool(name="ips", bufs=2, space="PSUM") as ips,
        ):
            f_sb = cpool.tile([128, 128], F16)
            g_sb = cpool.tile([128, 128], F16)
            w_sb = cpool.tile([128, NK, 128], F16)
            nc.gpsimd.dma_start(f_sb[:], fmat[:])
            nc.gpsimd.dma_start(g_sb[:], gmat[:])
            nc.gpsimd.dma_start(w_sb[:].rearrange("p k m -> p (k m)"),
                                wmat[:])

            if WARMUP:
                wps = fps.tile([128, 2, 8, 2, NA], F32, tag="fps")
                for i in range(WARMUP):
                    nc.tensor.matmul(
                        wps[:, i % 2, 0:2, :, :].rearrange(
                            "p w2 wp a -> p (w2 wp a)"),
                        f_sb[:], f_sb[:], skip_group_check=True)

            CW = BW // IN_SPLIT          # w per input chunk
            NFT = BW // 32               # fps tiles per window (2 mm each)
            MKT = max(1, 512 // BW)      # bins per mps bank-tile
            NMT = NK // MKT              # mps tiles per window
            NIV = W2 // 8                # inv matmuls per window
            for _ in range(reps):
                for h in range(NH):
                    # ---- input DMA ----
                    xh = []
                    for s in range(IN_SPLIT):
                        xt = px.tile([128, 32, CW], F16, tag="x")
                        nc.sync.dma_start(
                            xt[:].rearrange("p a w -> p (a w)"), x_in[h, s])
                        xh.append(xt)

                    # ---- fwd: BW/16 matmuls into 2-bank ring tiles ----
                    s_sb = ps_.tile([128, W2, NA], F32, tag="s")  # packed
                    s_v = s_sb[:].bitcast(F16).rearrange(
                        "p w2 (a wp) -> p w2 a wp", wp=2)
                    for q in range(NFT):
                        psf = fps.tile([128, 2, 8, 2, NA], F32, tag="fps")
                        for j in range(2):
                            wc = q * 2 + j           # w16-chunk in window
                            ws, wl = divmod(wc * 16, CW)
                            rhs = xh[ws][:, :, wl:wl + 16].rearrange(
                                "p a w -> p w a")
                            nc.tensor.matmul(
                                psf[:, j].rearrange(
                                    "p w2 wp a -> p (w2 wp) a"),
                                f_sb[:], rhs)
                        ev(EV_FWD[(h * NFT + q) % len(EV_FWD)])(
                            s_v[:, 16 * q:16 * q + 16, :, :],
                            psf[:].rearrange("p j w2 wp a -> p (j w2) a wp"))

                    # ---- T1: u32 StreamTranspose (swap a <-> quarter) ----
                    s2 = ps2_.tile([128, W2, NA], F32, tag="s2")
                    nc.vector.transpose(s2[:], s_sb[:])
                    s2_v = s2[:].bitcast(F16).rearrange(
                        "p w2 (k wp) -> p w2 k wp", wp=2)

                    # ---- mid: 32 per-bin matmuls, fp16 ----
                    o_sb = po_.tile([128, W2, NK], F32, tag="o")  # packed
                    o_v = o_sb[:].bitcast(F16).rearrange(
                        "p w2 (k wp) -> p w2 k wp", wp=2)
                    for q in range(NMT):
                        psm = mps.tile([128, MKT, W2, 2], F32, tag="mps")
                        for kk in range(MKT):
                            k = q * MKT + kk
                            nc.tensor.matmul(
                                psm[:, kk], w_sb[:, k, :], s2_v[:, :, k, :],
                                skip_group_check=True)
                        ev(EV_MID[(h * NMT + q) % len(EV_MID)])(
                            o_v[:, :, MKT * q:MKT * q + MKT, :].rearrange(
                                "p w2 k wp -> p k w2 wp"),
                            psm[:])

                    # ---- T2: u32 StreamTranspose (swap a_i <-> quarter) ----
                    v_sb = pv_.tile([128, W2, NK], F32, tag="v")
                    nc.vector.transpose(v_sb[:], o_sb[:])
                    v_v = v_sb[:].bitcast(F16).rearrange(
                        "p w2 (a wp) -> p w2 a wp", wp=2)

                    # ---- inv: matmuls into 1-bank ring tiles; evac to y ----
                    yt = py_.tile([128, W2, NA, 2], F16, tag="y")
                    for q in range(NIV):
                        w2c = q * 8
                        psi = ips.tile([128, 8, NA, 2], F32, tag="ips")
                        nc.tensor.matmul(
                            psi[:].rearrange("p w2 a wp -> p (w2 a wp)"),
                            g_sb[:], v_v[:, w2c:w2c + 8, :, :])
                        ev(EV_INV[(h * NIV + q) % len(EV_INV)])(
                            yt[:, w2c:w2c + 8, :, :], psi[:])
                    nc.gpsimd.dma_start(
                        y_out[h], yt[:].rearrange("p w2 a wp -> p (w2 a wp)"))

    nc.compile()
    return nc


_NC_CACHE = {}


def _inputs(x, W):
    fmat = make_fmat()
    gmat = make_gmat()
    wmat = make_wmats(np.asarray(W, np.float32)).reshape(128, NK * 128)
    xp = prep_x(np.ascontiguousarray(np.asarray(x, np.float32)))
    return [
        {"x": xp[c], "fmat": fmat, "gmat": gmat, "wmat": wmat}
        for c in range(N_CORES)
    ]


def run(x, W, reps=1):
    if reps not in _NC_CACHE:
        _NC_CACHE[reps] = build_nc(reps)
    res = run_bass_kernel_spmd(_NC_CACHE[reps], _inputs(x, W),
                               list(range(N_CORES)))
    return post_y([res.results[c]["y"] for c in range(N_CORES)])


def kernel(x, W):
    if 1 not in _NC_CACHE:
        _NC_CACHE[1] = build_nc(reps=1)
    res = run_bass_kernel_spmd(nc=_NC_CACHE[1], in_maps=_inputs(x, W),
                               core_ids=list(range(N_CORES)))
    return post_y([res.results[c]["y"] for c in range(N_CORES)])


# revision 22
# speedup vs baseline: 1.1680x; 1.1680x over previous
"""Self-contained Trainium2 kernel: block-circulant FFT linear layer.

out = ifft(fft(x_blocks) * fft(W)).real summed over input blocks, computed as
three PE matmul stages (real-FFT basis, per-frequency block matmul, inverse
real-FFT).  v3 design:

- fwd stage in fp32r (full PE rate at moving free dim 512); spectral data is
  cast to bf16 at PSUM evacuation; mid + inv stages run bf16 x bf16
  (1 cycle/row at any free dim -> small 128-batch windows pipeline finely)
- the two inter-stage shuffles are DVE 32x32 StreamTransposes on *u32-packed
  bf16 pairs* (two batch rows per element), halving DVE transpose work
- one shared 4-bank PSUM ring for all three stages -> few, large (2048-elem)
  evacuation copies, split between ACT and DVE
- flat per-partition-contiguous DMA layouts (128 descriptors per transfer);
  inputs on the sync HWDGE ring, outputs + consts on the gpsimd SWDGE ring.

SPMD over 8 NeuronCores (batch-sharded), 512 batch rows per core.

kernel(x, W): x [4096, 4096] f32, W [64, 64, 64] f32 -> [4096, 4096] f32.
"""
import numpy as np
import ml_dtypes
import concourse.bass as bass
import concourse.bacc as bacc
import concourse.mybir as mybir
import concourse.tile as tile
from concourse.tile import add_dep_helper
from concourse.bass_utils import run_bass_kernel_spmd

N_CORES = 8
B, IN, OUT, BS = 4096, 4096, 4096, 64
BC = B // N_CORES            # 512 batch rows per core
NK = 32                      # bin tiles (tile 0 carries bins 0 and 32)
NA = 32                      # jpair / ipair tiles
F32 = mybir.dt.float32
F32R = mybir.dt.float32r
BF16 = mybir.dt.bfloat16
F16 = mybir.dt.float16

# tunables
BW = 64                      # batch window
NH = BC // BW                # windows per core
W2 = BW // 2                 # u32-packed pair count per window
IN_SPLIT = 1                 # input DMA chunks per window
OUT_SPLIT = 1                # output DMA chunks per window
WARMUP = 64                  # PE warmup matmuls before first window
# evac engine per psum tile: 'v' = DVE, 's' = ACT
EV_FWD = ("s", "s", "s", "v")
EV_MID = ("s", "s", "v", "s")
EV_INV = ("v", "s", "v", "s")


# ---------------- host-side constant matrices ----------------

def make_fmat():
    t = np.arange(BS)[:, None]
    c = np.arange(BS)[None, :]
    k = np.where(c <= 32, c, c - 32)
    ang = 2 * np.pi * k * t / BS
    F = np.where(c <= 32, np.cos(ang), np.sin(ang))
    bd = np.zeros((128, 128), np.float32)
    bd[:64, :64] = F
    bd[64:, 64:] = F
    return bd.astype(np.float16)


def make_gmat():
    tau = np.arange(BS)[None, :]
    c = np.arange(BS)[:, None]
    k = np.where(c <= 32, c, c - 32)
    ang = 2 * np.pi * k * tau / BS
    base = np.where(c <= 32, np.cos(ang), np.sin(ang))
    scale = np.where((c % 32) == 0, 1.0 / BS, 2.0 / BS)
    G = base * scale
    bd = np.zeros((128, 128), np.float32)
    bd[:64, :64] = G
    bd[64:, 64:] = G
    return bd.astype(np.float16)


def make_wmats(W):
    s = np.arange(BS)
    k = np.arange(33)
    ang = 2 * np.pi * k[:, None] * s[None, :] / BS
    wr = np.einsum("ijs,ks->ijk", W, np.cos(ang))
    wi = np.einsum("ijs,ks->ijk", W, np.sin(ang))
    M = np.zeros((NK, 128, 128), np.float32)
    icol = np.empty(64, np.int64)
    for i in range(64):
        a, par = divmod(i, 2)
        icol[i] = 64 * par + a
    for kk in range(NK):
        if kk == 0:
            W32 = wr[:, :, 32]
        Wr, Wi = wr[:, :, kk], wi[:, :, kk]
        for par_j in range(2):
            jrows = np.arange(32) * 2 + par_j
            rre = 64 * par_j + np.arange(32)
            rim = rre + 32
            for i in range(64):
                cre = icol[i]
                cim = cre + 32
                if kk == 0:
                    M[0, rre, cre] = wr[i, jrows, 0]
                    M[0, rim, cim] = W32[i, jrows]
                else:
                    M[kk, rre, cre] = Wr[i, jrows]
                    M[kk, rim, cre] = -Wi[i, jrows]
                    M[kk, rre, cim] = Wi[i, jrows]
                    M[kk, rim, cim] = Wr[i, jrows]
    # device layout: [row r, tile k, col m]
    return np.ascontiguousarray(M.transpose(1, 0, 2)).astype(np.float16)


def prep_x(x):
    """[B, 4096] -> per-core [NH, IN_SPLIT, 128, 32*BW/IN_SPLIT] flat;
    p = par*64+t, j = 2a+par, w local within window."""
    xr = x.reshape(N_CORES, NH, BW, 32, 2, 64)  # [c, h, w, a, par, t]
    xp = xr.transpose(0, 1, 4, 5, 3, 2)  # [c, h, par, t, a, w]
    xp = xp.reshape(N_CORES, NH, 128, 32, IN_SPLIT, BW // IN_SPLIT)
    xp = xp.transpose(0, 1, 4, 2, 3, 5)  # [c, h, s, p, a, w]
    return np.ascontiguousarray(xp).astype(np.float16).reshape(
        N_CORES, NH, IN_SPLIT, 128, 32 * BW // IN_SPLIT)


def post_y(ys):
    """per-core [NH, 128, W2*32*2] -> [B, 4096]; p = par*64+tau, i = 2a+par,
    batch row = c*BC + h*BW + 2*w2 + wp."""
    y = np.stack(ys).astype(np.float32)  # [c, NH, 128, W2*NA*2]
    y = y.reshape(N_CORES, NH, 2, 64, W2, NA, 2)  # [c,h,par,tau,w2,a,wp]
    y = y.transpose(0, 1, 4, 6, 5, 2, 3)  # [c,h,w2,wp,a,par,tau]
    return np.ascontiguousarray(y).reshape(B, OUT)


# ---------------- device kernel ----------------

def build_nc(reps=1):
    nc = bacc.Bacc("TRN2", target_bir_lowering=False, debug=False,
                   num_devices=N_CORES, dynamic_dma_scratch_size=8192)
    XS = 32 * BW // IN_SPLIT
    x_in = nc.dram_tensor("x", [NH, IN_SPLIT, 128, XS], F16,
                          kind="ExternalInput")
    fmat = nc.dram_tensor("fmat", [128, 128], F16, kind="ExternalInput")
    gmat = nc.dram_tensor("gmat", [128, 128], F16, kind="ExternalInput")
    wmat = nc.dram_tensor("wmat", [2, 128, NK // 2 * 128], F16,
                          kind="ExternalInput")
    y_out = nc.dram_tensor("y", [NH, 128, W2 * NA * 2], F16,
                           kind="ExternalOutput")

    def ev(engine):
        return nc.vector.tensor_copy if engine == "v" else nc.scalar.copy

    with tile.TileContext(nc) as tc:
        with (
            tc.tile_pool(name="consts", bufs=1) as cpool,
            tc.tile_pool(name="px", bufs=8) as px,    # x in chunks
            tc.tile_pool(name="ps_", bufs=2) as ps_,  # s_sb (packed bf16)
            tc.tile_pool(name="ps2", bufs=2) as ps2_,
            tc.tile_pool(name="po", bufs=2) as po_,
            tc.tile_pool(name="pv", bufs=2) as pv_,
            tc.tile_pool(name="py", bufs=3) as py_,   # y windows
            tc.tile_pool(name="fps", bufs=2, space="PSUM") as fps,
            tc.tile_pool(name="mps", bufs=2, space="PSUM") as mps,
            tc.tile_pool(name="ips", bufs=2, space="PSUM") as ips,
        ):
            f_sb = cpool.tile([128, 128], F16)
            g_sb = cpool.tile([128, 128], F16)
            w_sb = cpool.tile([128, NK, 128], F16)
            nc.sync.dma_start(f_sb[:], fmat[:])
            nc.sync.dma_start(g_sb[:], gmat[:])
            for wh in range(2):
                nc.gpsimd.dma_start(
                    w_sb[:, wh * NK // 2:(wh + 1) * NK // 2, :].rearrange(
                        "p k m -> p (k m)"),
                    wmat[wh])

            if WARMUP:
                wps = fps.tile([128, 2, 8, 2, NA], F32, tag="fps")
                for i in range(WARMUP):
                    nc.tensor.matmul(
                        wps[:, i % 2, 0:2, :, :].rearrange(
                            "p w2 wp a -> p (w2 wp a)"),
                        f_sb[:], f_sb[:], skip_group_check=True)

            CW = BW // IN_SPLIT          # w per input chunk
            NFT = BW // 32               # fps tiles per window (2 mm each)
            MKT = max(1, 512 // BW)      # bins per mps bank-tile
            NMT = NK // MKT              # mps tiles per window
            NIV = W2 // 8                # inv matmuls per window
            for _ in range(reps):
                for h in range(NH):
                    # ---- input DMA ----
                    xh = []
                    for s in range(IN_SPLIT):
                        xt = px.tile([128, 32, CW], F16, tag="x")
                        nc.sync.dma_start(
                            xt[:].rearrange("p a w -> p (a w)"), x_in[h, s])
                        xh.append(xt)

                    # ---- fwd: BW/16 matmuls into 2-bank ring tiles ----
                    s_sb = ps_.tile([128, W2, NA], F32, tag="s")  # packed
                    s_v = s_sb[:].bitcast(F16).rearrange(
                        "p w2 (a wp) -> p w2 a wp", wp=2)
                    for q in range(NFT):
                        psf = fps.tile([128, 2, 8, 2, NA], F32, tag="fps")
                        for j in range(2):
                            wc = q * 2 + j           # w16-chunk in window
                            ws, wl = divmod(wc * 16, CW)
                            rhs = xh[ws][:, :, wl:wl + 16].rearrange(
                                "p a w -> p w a")
                            nc.tensor.matmul(
                                psf[:, j].rearrange(
                                    "p w2 wp a -> p (w2 wp) a"),
                                f_sb[:], rhs)
                        ev(EV_FWD[(h * NFT + q) % len(EV_FWD)])(
                            s_v[:, 16 * q:16 * q + 16, :, :],
                            psf[:].rearrange("p j w2 wp a -> p (j w2) a wp"))

                    # ---- T1: u32 StreamTranspose (swap a <-> quarter) ----
                    s2 = ps2_.tile([128, W2, NA], F32, tag="s2")
                    nc.vector.transpose(s2[:], s_sb[:])
                    s2_v = s2[:].bitcast(F16).rearrange(
                        "p w2 (k wp) -> p w2 k wp", wp=2)

                    # ---- mid: 32 per-bin matmuls, fp16 ----
                    o_sb = po_.tile([128, W2, NK], F32, tag="o")  # packed
                    o_v = o_sb[:].bitcast(F16).rearrange(
                        "p w2 (k wp) -> p w2 k wp", wp=2)
                    for q in range(NMT):
                        psm = mps.tile([128, MKT, W2, 2], F32, tag="mps")
                        for kk in range(MKT):
                            k = q * MKT + kk
                            nc.tensor.matmul(
                                psm[:, kk], w_sb[:, k, :], s2_v[:, :, k, :],
                                skip_group_check=True)
                        ev(EV_MID[(h * NMT + q) % len(EV_MID)])(
                            o_v[:, :, MKT * q:MKT * q + MKT, :].rearrange(
                                "p w2 k wp -> p k w2 wp"),
                            psm[:])

                    # ---- T2: u32 StreamTranspose (swap a_i <-> quarter) ----
                    v_sb = pv_.tile([128, W2, NK], F32, tag="v")
                    nc.vector.transpose(v_sb[:], o_sb[:])
                    v_v = v_sb[:].bitcast(F16).rearrange(
                        "p w2 (a wp) -> p w2 a wp", wp=2)

                    # ---- inv: matmuls into 1-bank ring tiles; evac to y ----
                    yt = py_.tile([128, W2, NA, 2], F16, tag="y")
                    for q in range(NIV):
                        w2c = q * 8
                        psi = ips.tile([128, 8, NA, 2], F32, tag="ips")
                        nc.tensor.matmul(
                            psi[:].rearrange("p w2 a wp -> p (w2 a wp)"),
                            g_sb[:], v_v[:, w2c:w2c + 8, :, :])
                        ev(EV_INV[(h * NIV + q) % len(EV_INV)])(
                            yt[:, w2c:w2c + 8, :, :], psi[:])
                    yf = yt[:].rearrange("p w2 a wp -> p (w2 a wp)")
                    last = h == NH - 1
                    osp = 2 if last else OUT_SPLIT
                    YH = W2 * NA * 2 // osp
                    for s in range(osp):
                        eng = nc.scalar if last else nc.gpsimd
                        eng.dma_start(y_out[h, :, s * YH:(s + 1) * YH],
                                      yf[:, s * YH:(s + 1) * YH])

    nc.compile()
    return nc


_NC_CACHE = {}


def _inputs(x, W):
    fmat = make_fmat()
    gmat = make_gmat()
    wmat = make_wmats(np.asarray(W, np.float32)).reshape(
        128, 2, NK // 2 * 128).transpose(1, 0, 2).copy()
    xp = prep_x(np.ascontiguousarray(np.asarray(x, np.float32)))
    return [
        {"x": xp[c], "fmat": fmat, "gmat": gmat, "wmat": wmat}
        for c in range(N_CORES)
    ]


def run(x, W, reps=1):
    if reps not in _NC_CACHE:
        _NC_CACHE[reps] = build_nc(reps)
    res = run_bass_kernel_spmd(_NC_CACHE[reps], _inputs(x, W),
                               list(range(N_CORES)))
    return post_y([res.results[c]["y"] for c in range(N_CORES)])


def kernel(x, W):
    if 1 not in _NC_CACHE:
        _NC_CACHE[1] = build_nc(reps=1)
    res = run_bass_kernel_spmd(nc=_NC_CACHE[1], in_maps=_inputs(x, W),
                               core_ids=list(range(N_CORES)))
    return post_y([res.results[c]["y"] for c in range(N_CORES)])


# revision 23
# speedup vs baseline: 1.2306x; 1.0536x over previous
"""Self-contained Trainium2 kernel: block-circulant FFT linear layer.

out = ifft(fft(x_blocks) * fft(W)).real summed over input blocks, computed as
three PE matmul stages (real-FFT basis, per-frequency block matmul, inverse
real-FFT).  v3 design:

- fwd stage in fp32r (full PE rate at moving free dim 512); spectral data is
  cast to bf16 at PSUM evacuation; mid + inv stages run bf16 x bf16
  (1 cycle/row at any free dim -> small 128-batch windows pipeline finely)
- the two inter-stage shuffles are DVE 32x32 StreamTransposes on *u32-packed
  bf16 pairs* (two batch rows per element), halving DVE transpose work
- one shared 4-bank PSUM ring for all three stages -> few, large (2048-elem)
  evacuation copies, split between ACT and DVE
- flat per-partition-contiguous DMA layouts (128 descriptors per transfer);
  inputs on the sync HWDGE ring, outputs + consts on the gpsimd SWDGE ring.

SPMD over 8 NeuronCores (batch-sharded), 512 batch rows per core.

kernel(x, W): x [4096, 4096] f32, W [64, 64, 64] f32 -> [4096, 4096] f32.
"""
import numpy as np
import ml_dtypes
import concourse.bass as bass
import concourse.bacc as bacc
import concourse.mybir as mybir
import concourse.tile as tile
from concourse.tile import add_dep_helper
from concourse.bass_utils import run_bass_kernel_spmd

N_CORES = 8
B, IN, OUT, BS = 4096, 4096, 4096, 64
BC = B // N_CORES            # 512 batch rows per core
NK = 32                      # bin tiles (tile 0 carries bins 0 and 32)
NA = 32                      # jpair / ipair tiles
F32 = mybir.dt.float32
F32R = mybir.dt.float32r
BF16 = mybir.dt.bfloat16
F16 = mybir.dt.float16

# tunables
BW = 64                      # batch window
NH = BC // BW                # windows per core
W2 = BW // 2                 # u32-packed pair count per window
IN_SPLIT = 1                 # input DMA chunks per window
OUT_SPLIT = 1                # output DMA chunks per window
WARMUP = 64                  # PE warmup matmuls before first window
# evac engine per psum tile: 'v' = DVE, 's' = ACT
EV_FWD = ("v", "s", "s", "s")
EV_MID = ("s", "v", "s", "s")
EV_INV = ("s", "v", "v", "s")


# ---------------- host-side constant matrices ----------------

def make_fmat():
    t = np.arange(BS)[:, None]
    c = np.arange(BS)[None, :]
    k = np.where(c <= 32, c, c - 32)
    ang = 2 * np.pi * k * t / BS
    F = np.where(c <= 32, np.cos(ang), np.sin(ang))
    bd = np.zeros((128, 128), np.float32)
    bd[:64, :64] = F
    bd[64:, 64:] = F
    return bd.astype(np.float16)


def make_gmat():
    tau = np.arange(BS)[None, :]
    c = np.arange(BS)[:, None]
    k = np.where(c <= 32, c, c - 32)
    ang = 2 * np.pi * k * tau / BS
    base = np.where(c <= 32, np.cos(ang), np.sin(ang))
    scale = np.where((c % 32) == 0, 1.0 / BS, 2.0 / BS)
    G = base * scale
    bd = np.zeros((128, 128), np.float32)
    bd[:64, :64] = G
    bd[64:, 64:] = G
    return bd.astype(np.float16)


def make_wmats(W):
    s = np.arange(BS)
    k = np.arange(33)
    ang = 2 * np.pi * k[:, None] * s[None, :] / BS
    wr = np.einsum("ijs,ks->ijk", W, np.cos(ang))
    wi = np.einsum("ijs,ks->ijk", W, np.sin(ang))
    M = np.zeros((NK, 128, 128), np.float32)
    icol = np.empty(64, np.int64)
    for i in range(64):
        a, par = divmod(i, 2)
        icol[i] = 64 * par + a
    for kk in range(NK):
        if kk == 0:
            W32 = wr[:, :, 32]
        Wr, Wi = wr[:, :, kk], wi[:, :, kk]
        for par_j in range(2):
            jrows = np.arange(32) * 2 + par_j
            rre = 64 * par_j + np.arange(32)
            rim = rre + 32
            for i in range(64):
                cre = icol[i]
                cim = cre + 32
                if kk == 0:
                    M[0, rre, cre] = wr[i, jrows, 0]
                    M[0, rim, cim] = W32[i, jrows]
                else:
                    M[kk, rre, cre] = Wr[i, jrows]
                    M[kk, rim, cre] = -Wi[i, jrows]
                    M[kk, rre, cim] = Wi[i, jrows]
                    M[kk, rim, cim] = Wr[i, jrows]
    # device layout: [row r, tile k, col m]
    return np.ascontiguousarray(M.transpose(1, 0, 2)).astype(np.float16)


def prep_x(x):
    """[B, 4096] -> per-core [NH, IN_SPLIT, 128, 32*BW/IN_SPLIT] flat;
    p = par*64+t, j = 2a+par, w local within window."""
    xr = x.reshape(N_CORES, NH, BW, 32, 2, 64)  # [c, h, w, a, par, t]
    xp = xr.transpose(0, 1, 4, 5, 3, 2)  # [c, h, par, t, a, w]
    xp = xp.reshape(N_CORES, NH, 128, 32, IN_SPLIT, BW // IN_SPLIT)
    xp = xp.transpose(0, 1, 4, 2, 3, 5)  # [c, h, s, p, a, w]
    return np.ascontiguousarray(xp).astype(np.float16).reshape(
        N_CORES, NH, IN_SPLIT, 128, 32 * BW // IN_SPLIT)


def post_y(ys):
    """per-core [NH, 128, W2*32*2] -> [B, 4096]; p = par*64+tau, i = 2a+par,
    batch row = c*BC + h*BW + 2*w2 + wp."""
    y = np.stack(ys).astype(np.float32)  # [c, NH, 128, W2*NA*2]
    y = y.reshape(N_CORES, NH, 2, 64, W2, NA, 2)  # [c,h,par,tau,w2,a,wp]
    y = y.transpose(0, 1, 4, 6, 5, 2, 3)  # [c,h,w2,wp,a,par,tau]
    return np.ascontiguousarray(y).reshape(B, OUT)


# ---------------- device kernel ----------------

def build_nc(reps=1):
    nc = bacc.Bacc("TRN2", target_bir_lowering=False, debug=False,
                   num_devices=N_CORES, dynamic_dma_scratch_size=8192)
    XS = 32 * BW // IN_SPLIT
    x_in = nc.dram_tensor("x", [NH, IN_SPLIT, 128, XS], F16,
                          kind="ExternalInput")
    fmat = nc.dram_tensor("fmat", [128, 128], F16, kind="ExternalInput")
    gmat = nc.dram_tensor("gmat", [128, 128], F16, kind="ExternalInput")
    wmat = nc.dram_tensor("wmat", [2, 128, NK // 2 * 128], F16,
                          kind="ExternalInput")
    y_out = nc.dram_tensor("y", [NH, 128, W2 * NA * 2], F16,
                           kind="ExternalOutput")

    def ev(engine):
        return nc.vector.tensor_copy if engine == "v" else nc.scalar.copy

    with tile.TileContext(nc) as tc:
        with (
            tc.tile_pool(name="consts", bufs=1) as cpool,
            tc.tile_pool(name="px", bufs=8) as px,    # x in chunks
            tc.tile_pool(name="ps_", bufs=2) as ps_,  # s_sb (packed bf16)
            tc.tile_pool(name="ps2", bufs=2) as ps2_,
            tc.tile_pool(name="po", bufs=2) as po_,
            tc.tile_pool(name="pv", bufs=2) as pv_,
            tc.tile_pool(name="py", bufs=3) as py_,   # y windows
            tc.tile_pool(name="fps", bufs=2, space="PSUM") as fps,
            tc.tile_pool(name="mps", bufs=2, space="PSUM") as mps,
            tc.tile_pool(name="ips", bufs=2, space="PSUM") as ips,
        ):
            f_sb = cpool.tile([128, 128], F16)
            g_sb = cpool.tile([128, 128], F16)
            w_sb = cpool.tile([128, NK, 128], F16)
            nc.sync.dma_start(f_sb[:], fmat[:])
            nc.sync.dma_start(g_sb[:], gmat[:])
            for wh in range(2):
                nc.gpsimd.dma_start(
                    w_sb[:, wh * NK // 2:(wh + 1) * NK // 2, :].rearrange(
                        "p k m -> p (k m)"),
                    wmat[wh])

            if WARMUP:
                wps = fps.tile([128, 2, 8, 2, NA], F32, tag="fps")
                for i in range(WARMUP):
                    nc.tensor.matmul(
                        wps[:, i % 2, 0:2, :, :].rearrange(
                            "p w2 wp a -> p (w2 wp a)"),
                        f_sb[:], f_sb[:], skip_group_check=True)

            CW = BW // IN_SPLIT          # w per input chunk
            NFT = BW // 32               # fps tiles per window (2 mm each)
            MKT = max(1, 512 // BW)      # bins per mps bank-tile
            NMT = NK // MKT              # mps tiles per window
            NIV = W2 // 8                # inv matmuls per window
            for _ in range(reps):
                for h in range(NH):
                    # ---- input DMA ----
                    xh = []
                    for s in range(IN_SPLIT):
                        xt = px.tile([128, 32, CW], F16, tag="x")
                        nc.sync.dma_start(
                            xt[:].rearrange("p a w -> p (a w)"), x_in[h, s])
                        xh.append(xt)

                    # ---- fwd: BW/16 matmuls into 2-bank ring tiles ----
                    s_sb = ps_.tile([128, W2, NA], F32, tag="s")  # packed
                    s_v = s_sb[:].bitcast(F16).rearrange(
                        "p w2 (a wp) -> p w2 a wp", wp=2)
                    for q in range(NFT):
                        psf = fps.tile([128, 2, 8, 2, NA], F32, tag="fps")
                        for j in range(2):
                            wc = q * 2 + j           # w16-chunk in window
                            ws, wl = divmod(wc * 16, CW)
                            rhs = xh[ws][:, :, wl:wl + 16].rearrange(
                                "p a w -> p w a")
                            nc.tensor.matmul(
                                psf[:, j].rearrange(
                                    "p w2 wp a -> p (w2 wp) a"),
                                f_sb[:], rhs)
                        ev(EV_FWD[(h * NFT + q) % len(EV_FWD)])(
                            s_v[:, 16 * q:16 * q + 16, :, :],
                            psf[:].rearrange("p j w2 wp a -> p (j w2) a wp"))

                    # ---- T1: u32 StreamTranspose (swap a <-> quarter) ----
                    s2 = ps2_.tile([128, W2, NA], F32, tag="s2")
                    nc.vector.transpose(s2[:], s_sb[:])
                    s2_v = s2[:].bitcast(F16).rearrange(
                        "p w2 (k wp) -> p w2 k wp", wp=2)

                    # ---- mid: 32 per-bin matmuls, fp16 ----
                    o_sb = po_.tile([128, W2, NK], F32, tag="o")  # packed
                    o_v = o_sb[:].bitcast(F16).rearrange(
                        "p w2 (k wp) -> p w2 k wp", wp=2)
                    for q in range(NMT):
                        psm = mps.tile([128, MKT, W2, 2], F32, tag="mps")
                        for kk in range(MKT):
                            k = q * MKT + kk
                            nc.tensor.matmul(
                                psm[:, kk], w_sb[:, k, :], s2_v[:, :, k, :],
                                skip_group_check=True)
                        ev(EV_MID[(h * NMT + q) % len(EV_MID)])(
                            o_v[:, :, MKT * q:MKT * q + MKT, :].rearrange(
                                "p w2 k wp -> p k w2 wp"),
                            psm[:])

                    # ---- T2: u32 StreamTranspose (swap a_i <-> quarter) ----
                    v_sb = pv_.tile([128, W2, NK], F32, tag="v")
                    nc.vector.transpose(v_sb[:], o_sb[:])
                    v_v = v_sb[:].bitcast(F16).rearrange(
                        "p w2 (a wp) -> p w2 a wp", wp=2)

                    # ---- inv: matmuls into 1-bank ring tiles; evac to y ----
                    yt = py_.tile([128, W2, NA, 2], F16, tag="y")
                    for q in range(NIV):
                        w2c = q * 8
                        psi = ips.tile([128, 8, NA, 2], F32, tag="ips")
                        nc.tensor.matmul(
                            psi[:].rearrange("p w2 a wp -> p (w2 a wp)"),
                            g_sb[:], v_v[:, w2c:w2c + 8, :, :])
                        ev(EV_INV[(h * NIV + q) % len(EV_INV)])(
                            yt[:, w2c:w2c + 8, :, :], psi[:])
                    yf = yt[:].rearrange("p w2 a wp -> p (w2 a wp)")
                    last = h == NH - 1
                    osp = 2 if last else OUT_SPLIT
                    YH = W2 * NA * 2 // osp
                    for s in range(osp):
                        eng = nc.scalar if last else nc.gpsimd
                        eng.dma_start(y_out[h, :, s * YH:(s + 1) * YH],
                                      yf[:, s * YH:(s + 1) * YH])

    nc.compile()
    return nc


_NC_CACHE = {}


def _inputs(x, W):
    fmat = make_fmat()
    gmat = make_gmat()
    wmat = make_wmats(np.asarray(W, np.float32)).reshape(
        128, 2, NK // 2 * 128).transpose(1, 0, 2).copy()
    xp = prep_x(np.ascontiguousarray(np.asarray(x, np.float32)))
    return [
        {"x": xp[c], "fmat": fmat, "gmat": gmat, "wmat": wmat}
        for c in range(N_CORES)
    ]


def run(x, W, reps=1):
    if reps not in _NC_CACHE:
        _NC_CACHE[reps] = build_nc(reps)
    res = run_bass_kernel_spmd(_NC_CACHE[reps], _inputs(x, W),
                               list(range(N_CORES)))
    return post_y([res.results[c]["y"] for c in range(N_CORES)])


def kernel(x, W):
    if 1 not in _NC_CACHE:
        _NC_CACHE[1] = build_nc(reps=1)
    res = run_bass_kernel_spmd(nc=_NC_CACHE[1], in_maps=_inputs(x, W),
                               core_ids=list(range(N_CORES)))
    return post_y([res.results[c]["y"] for c in range(N_CORES)])


# revision 24
# speedup vs baseline: 1.4048x; 1.1415x over previous
"""Self-contained Trainium2 kernel: block-circulant FFT linear layer.

out = ifft(fft(x_blocks) * fft(W)).real summed over input blocks, computed as
three PE matmul stages (real-FFT basis, per-frequency block matmul, inverse
real-FFT).  v3 design:

- fp16 end-to-end (x and y converted on the host, constants fp16): all PE
  stages run at 1 cycle/row, and DMA traffic halves to ~9 MB/core
- the two inter-stage shuffles are DVE 32x32 StreamTransposes on *u32-packed
  fp16 pairs* (two batch rows per element), halving DVE transpose work
- PSUM evacuations (with f32 -> fp16 cast) split between ACT and DVE by a
  tuned per-tile pattern; per-stage double-buffered PSUM rings
- small 64-batch windows pipeline the 8-window stream finely; flat
  per-partition-contiguous DMA layouts (128 descriptors per transfer);
  inputs + consts on the sync HWDGE ring, outputs on the gpsimd SWDGE ring
  (wmat on SWDGE) so they never head-of-line block the input stream.

SPMD over 8 NeuronCores (batch-sharded), 512 batch rows per core.

kernel(x, W): x [4096, 4096] f32, W [64, 64, 64] f32 -> [4096, 4096] f32.
"""
import numpy as np
import concourse.bacc as bacc
import concourse.mybir as mybir
import concourse.tile as tile
from concourse.bass_utils import run_bass_kernel_spmd

N_CORES = 8
B, IN, OUT, BS = 4096, 4096, 4096, 64
BC = B // N_CORES            # 512 batch rows per core
NK = 32                      # bin tiles (tile 0 carries bins 0 and 32)
NA = 32                      # jpair / ipair tiles
F32 = mybir.dt.float32
F32R = mybir.dt.float32r
BF16 = mybir.dt.bfloat16
F16 = mybir.dt.float16

# tunables
BW = 64                      # batch window
NH = BC // BW                # windows per core
W2 = BW // 2                 # u32-packed pair count per window
IN_SPLIT = 1                 # input DMA chunks per window
OUT_SPLIT = 1                # output DMA chunks per window
WARMUP = 64                  # PE warmup matmuls before first window
# evac engine per psum tile: 'v' = DVE, 's' = ACT
EV_FWD = ("v", "s", "s", "s")
EV_MID = ("s", "v", "s", "s")
EV_INV = ("s", "v", "v", "s")


# ---------------- host-side constant matrices ----------------

def make_fmat():
    t = np.arange(BS)[:, None]
    c = np.arange(BS)[None, :]
    k = np.where(c <= 32, c, c - 32)
    ang = 2 * np.pi * k * t / BS
    F = np.where(c <= 32, np.cos(ang), np.sin(ang))
    bd = np.zeros((128, 128), np.float32)
    bd[:64, :64] = F
    bd[64:, 64:] = F
    return bd.astype(np.float16)


def make_gmat():
    tau = np.arange(BS)[None, :]
    c = np.arange(BS)[:, None]
    k = np.where(c <= 32, c, c - 32)
    ang = 2 * np.pi * k * tau / BS
    base = np.where(c <= 32, np.cos(ang), np.sin(ang))
    scale = np.where((c % 32) == 0, 1.0 / BS, 2.0 / BS)
    G = base * scale
    bd = np.zeros((128, 128), np.float32)
    bd[:64, :64] = G
    bd[64:, 64:] = G
    return bd.astype(np.float16)


def make_wmats(W):
    s = np.arange(BS)
    k = np.arange(33)
    ang = 2 * np.pi * k[:, None] * s[None, :] / BS
    wr = np.einsum("ijs,ks->ijk", W, np.cos(ang))
    wi = np.einsum("ijs,ks->ijk", W, np.sin(ang))
    M = np.zeros((NK, 128, 128), np.float32)
    icol = np.empty(64, np.int64)
    for i in range(64):
        a, par = divmod(i, 2)
        icol[i] = 64 * par + a
    for kk in range(NK):
        if kk == 0:
            W32 = wr[:, :, 32]
        Wr, Wi = wr[:, :, kk], wi[:, :, kk]
        for par_j in range(2):
            jrows = np.arange(32) * 2 + par_j
            rre = 64 * par_j + np.arange(32)
            rim = rre + 32
            for i in range(64):
                cre = icol[i]
                cim = cre + 32
                if kk == 0:
                    M[0, rre, cre] = wr[i, jrows, 0]
                    M[0, rim, cim] = W32[i, jrows]
                else:
                    M[kk, rre, cre] = Wr[i, jrows]
                    M[kk, rim, cre] = -Wi[i, jrows]
                    M[kk, rre, cim] = Wi[i, jrows]
                    M[kk, rim, cim] = Wr[i, jrows]
    # device layout: [row r, tile k, col m]
    return np.ascontiguousarray(M.transpose(1, 0, 2)).astype(np.float16)


def prep_x(x):
    """[B, 4096] -> per-core [NH, IN_SPLIT, 128, 32*BW/IN_SPLIT] flat;
    p = par*64+t, j = 2a+par, w local within window."""
    xr = x.reshape(N_CORES, NH, BW, 32, 2, 64)  # [c, h, w, a, par, t]
    xp = xr.transpose(0, 1, 4, 5, 3, 2)  # [c, h, par, t, a, w]
    xp = xp.reshape(N_CORES, NH, 128, 32, IN_SPLIT, BW // IN_SPLIT)
    xp = xp.transpose(0, 1, 4, 2, 3, 5)  # [c, h, s, p, a, w]
    return np.ascontiguousarray(xp).astype(np.float16).reshape(
        N_CORES, NH, IN_SPLIT, 128, 32 * BW // IN_SPLIT)


def post_y(ys):
    """per-core [NH, 128, W2*32*2] -> [B, 4096]; p = par*64+tau, i = 2a+par,
    batch row = c*BC + h*BW + 2*w2 + wp."""
    y = np.stack(ys).astype(np.float32)  # [c, NH, 128, W2*NA*2]
    y = y.reshape(N_CORES, NH, 2, 64, W2, NA, 2)  # [c,h,par,tau,w2,a,wp]
    y = y.transpose(0, 1, 4, 6, 5, 2, 3)  # [c,h,w2,wp,a,par,tau]
    return np.ascontiguousarray(y).reshape(B, OUT)


# ---------------- device kernel ----------------

def build_nc(reps=1):
    nc = bacc.Bacc("TRN2", target_bir_lowering=False, debug=False,
                   num_devices=N_CORES, dynamic_dma_scratch_size=8192)
    XS = 32 * BW // IN_SPLIT
    x_in = nc.dram_tensor("x", [NH, IN_SPLIT, 128, XS], F16,
                          kind="ExternalInput")
    fmat = nc.dram_tensor("fmat", [128, 128], F16, kind="ExternalInput")
    gmat = nc.dram_tensor("gmat", [128, 128], F16, kind="ExternalInput")
    wmat = nc.dram_tensor("wmat", [2, 128, NK // 2 * 128], F16,
                          kind="ExternalInput")
    y_out = nc.dram_tensor("y", [NH, 128, W2 * NA * 2], F16,
                           kind="ExternalOutput")

    def ev(engine):
        return nc.vector.tensor_copy if engine == "v" else nc.scalar.copy

    with tile.TileContext(nc) as tc:
        with (
            tc.tile_pool(name="consts", bufs=1) as cpool,
            tc.tile_pool(name="px", bufs=8) as px,    # x in chunks
            tc.tile_pool(name="ps_", bufs=2) as ps_,  # s_sb (packed bf16)
            tc.tile_pool(name="ps2", bufs=2) as ps2_,
            tc.tile_pool(name="po", bufs=2) as po_,
            tc.tile_pool(name="pv", bufs=2) as pv_,
            tc.tile_pool(name="py", bufs=3) as py_,   # y windows
            tc.tile_pool(name="fps", bufs=2, space="PSUM") as fps,
            tc.tile_pool(name="mps", bufs=2, space="PSUM") as mps,
            tc.tile_pool(name="ips", bufs=2, space="PSUM") as ips,
        ):
            f_sb = cpool.tile([128, 128], F16)
            g_sb = cpool.tile([128, 128], F16)
            w_sb = cpool.tile([128, NK, 128], F16)
            nc.sync.dma_start(f_sb[:], fmat[:])
            nc.sync.dma_start(g_sb[:], gmat[:])
            for wh in range(2):
                nc.gpsimd.dma_start(
                    w_sb[:, wh * NK // 2:(wh + 1) * NK // 2, :].rearrange(
                        "p k m -> p (k m)"),
                    wmat[wh])

            if WARMUP:
                wps = fps.tile([128, 2, 8, 2, NA], F32, tag="fps")
                for i in range(WARMUP):
                    nc.tensor.matmul(
                        wps[:, i % 2, 0:2, :, :].rearrange(
                            "p w2 wp a -> p (w2 wp a)"),
                        f_sb[:], f_sb[:], skip_group_check=True)

            CW = BW // IN_SPLIT          # w per input chunk
            NFT = BW // 32               # fps tiles per window (2 mm each)
            MKT = max(1, 512 // BW)      # bins per mps bank-tile
            NMT = NK // MKT              # mps tiles per window
            NIV = W2 // 8                # inv matmuls per window
            for _ in range(reps):
                for h in range(NH):
                    # ---- input DMA ----
                    xh = []
                    for s in range(IN_SPLIT):
                        xt = px.tile([128, 32, CW], F16, tag="x")
                        nc.sync.dma_start(
                            xt[:].rearrange("p a w -> p (a w)"), x_in[h, s])
                        xh.append(xt)

                    # ---- fwd: BW/16 matmuls into 2-bank ring tiles ----
                    s_sb = ps_.tile([128, W2, NA], F32, tag="s")  # packed
                    s_v = s_sb[:].bitcast(F16).rearrange(
                        "p w2 (a wp) -> p w2 a wp", wp=2)
                    for q in range(NFT):
                        psf = fps.tile([128, 2, 8, 2, NA], F32, tag="fps")
                        for j in range(2):
                            wc = q * 2 + j           # w16-chunk in window
                            ws, wl = divmod(wc * 16, CW)
                            rhs = xh[ws][:, :, wl:wl + 16].rearrange(
                                "p a w -> p w a")
                            nc.tensor.matmul(
                                psf[:, j].rearrange(
                                    "p w2 wp a -> p (w2 wp) a"),
                                f_sb[:], rhs)
                        ev(EV_FWD[(h * NFT + q) % len(EV_FWD)])(
                            s_v[:, 16 * q:16 * q + 16, :, :],
                            psf[:].rearrange("p j w2 wp a -> p (j w2) a wp"))

                    # ---- T1: u32 StreamTranspose (swap a <-> quarter) ----
                    s2 = ps2_.tile([128, W2, NA], F32, tag="s2")
                    nc.vector.transpose(s2[:], s_sb[:])
                    s2_v = s2[:].bitcast(F16).rearrange(
                        "p w2 (k wp) -> p w2 k wp", wp=2)

                    # ---- mid: 32 per-bin matmuls, fp16 ----
                    o_sb = po_.tile([128, W2, NK], F32, tag="o")  # packed
                    o_v = o_sb[:].bitcast(F16).rearrange(
                        "p w2 (k wp) -> p w2 k wp", wp=2)
                    for q in range(NMT):
                        psm = mps.tile([128, MKT, W2, 2], F32, tag="mps")
                        for kk in range(MKT):
                            k = q * MKT + kk
                            nc.tensor.matmul(
                                psm[:, kk], w_sb[:, k, :], s2_v[:, :, k, :],
                                skip_group_check=True)
                        ev(EV_MID[(h * NMT + q) % len(EV_MID)])(
                            o_v[:, :, MKT * q:MKT * q + MKT, :].rearrange(
                                "p w2 k wp -> p k w2 wp"),
                            psm[:])

                    # ---- T2: u32 StreamTranspose (swap a_i <-> quarter) ----
                    v_sb = pv_.tile([128, W2, NK], F32, tag="v")
                    nc.vector.transpose(v_sb[:], o_sb[:])
                    v_v = v_sb[:].bitcast(F16).rearrange(
                        "p w2 (a wp) -> p w2 a wp", wp=2)

                    # ---- inv: matmuls into 1-bank ring tiles; evac to y ----
                    yt = py_.tile([128, W2, NA, 2], F16, tag="y")
                    for q in range(NIV):
                        w2c = q * 8
                        psi = ips.tile([128, 8, NA, 2], F32, tag="ips")
                        nc.tensor.matmul(
                            psi[:].rearrange("p w2 a wp -> p (w2 a wp)"),
                            g_sb[:], v_v[:, w2c:w2c + 8, :, :])
                        ev(EV_INV[(h * NIV + q) % len(EV_INV)])(
                            yt[:, w2c:w2c + 8, :, :], psi[:])
                    yf = yt[:].rearrange("p w2 a wp -> p (w2 a wp)")
                    last = h == NH - 1
                    osp = 2 if last else OUT_SPLIT
                    YH = W2 * NA * 2 // osp
                    for s in range(osp):
                        eng = nc.scalar if last else nc.gpsimd
                        eng.dma_start(y_out[h, :, s * YH:(s + 1) * YH],
                                      yf[:, s * YH:(s + 1) * YH])

    nc.compile()
    return nc


_NC_CACHE = {}


def _inputs(x, W):
    fmat = make_fmat()
    gmat = make_gmat()
    wmat = make_wmats(np.asarray(W, np.float32)).reshape(
        128, 2, NK // 2 * 128).transpose(1, 0, 2).copy()
    xp = prep_x(np.ascontiguousarray(np.asarray(x, np.float32)))
    return [
        {"x": xp[c], "fmat": fmat, "gmat": gmat, "wmat": wmat}
        for c in range(N_CORES)
    ]


def run(x, W, reps=1):
    if reps not in _NC_CACHE:
        _NC_CACHE[reps] = build_nc(reps)
    res = run_bass_kernel_spmd(_NC_CACHE[reps], _inputs(x, W),
                               list(range(N_CORES)))
    return post_y([res.results[c]["y"] for c in range(N_CORES)])


def kernel(x, W):
    if 1 not in _NC_CACHE:
        _NC_CACHE[1] = build_nc(reps=1)
    res = run_bass_kernel_spmd(nc=_NC_CACHE[1], in_maps=_inputs(x, W),
                               core_ids=list(range(N_CORES)))
    return post_y([res.results[c]["y"] for c in range(N_CORES)])
